# revision 3
# baseline (speedup 1.0000x reference)
"""Trainium2 Bass kernel for nn_ErdosLoss (graph loss function).

Math (reference reformulated, validated to ~1e-6 rel err):
  penalty:  log_score = scatter_add(log(1 - p + 1e-6), tgt)   over N nodes
            loss2 = mean(exp(log_score)) * 9600
  loss3:    p @ triu(H H^T, 1) @ p^T  ==  (||s||^2 - sum_e d_e p_e^2) / 2
            where s = scatter_add(p, tgt) + scatter_add(p * (1-m), src),
            m_e = (src_e == tgt_e)  (H rows are node *sets*: self-loops get a
            single 1), d_e = 2 - m_e.
  out = loss2 + 200 * loss3 / num_graphs,  num_graphs = max(batch) + 1.

Device strategy (8 NeuronCores, SPMD):
  - Edges sharded 750/core (padded to 6 tiles of 128).
  - Scatter-add = one-hot matmul with a 2-level node decomposition:
    node = 128*hi + lo (N padded to 4096 = 128*32).  Per edge tile build
    lo-one-hot [128e,128lo] (tensor_scalar is_equal vs an iota row) and a
    value-weighted hi-one-hot [128e,32hi] (fused is_equal*mult), then
    TensorE matmul accumulates [128lo, 32hi] node partials in PSUM.
  - Per-core partials packed as [128, 65] (log_score | s | dp2 rowsum) and
    combined with a single AllReduce; every core redundantly computes the
    final scalar (exp/square row-sums, ones-matmul partition reduce,
    max(batch)+1 on device) and writes out [1,1].
"""

import numpy as np

import concourse.bass as bass
import concourse.bacc as bacc
import concourse.mybir as mybir
import concourse.tile as tile
import concourse.bass_isa as bass_isa
from concourse import bass_utils

F32 = mybir.dt.float32
ALU = mybir.AluOpType
ACT = mybir.ActivationFunctionType

N_NODES = 4000
N_EDGES = 6000
N_CORES = 8
N_PAD = 4096          # 128 * 32
HI = 32               # node hi-digits
LO = 128              # node lo-digits
PENALTY_SCALE = 16 * 200 * 3   # 9600
PAD_NODES = N_PAD - N_NODES    # 96 padded nodes, each contributes exp(0)=1

EPC = N_EDGES // N_CORES       # 750 edges per core
TPC = (EPC + 127) // 128       # 6 edge tiles per core
EPAD = TPC * 128               # 768

# edata column layout: 7 fields x TPC columns
_F_TLO, _F_THI, _F_ULO, _F_UHI, _F_TF, _F_UF, _F_P = range(7)


def _build_program(tiles_per_core: int, use_collective: bool):
    """Build the SPMD Bass program (same program on all cores)."""
    T = tiles_per_core
    nc = bacc.Bacc(
        "TRN2",
        target_bir_lowering=False,
        debug=False,
        num_devices=N_CORES if use_collective else 1,
    )

    edata = nc.dram_tensor("edata", [128, 7 * T], F32, kind="ExternalInput").ap()
    iota128 = nc.dram_tensor("iota128", [128, LO], F32, kind="ExternalInput").ap()
    iota32 = nc.dram_tensor("iota32", [128, HI], F32, kind="ExternalInput").ap()
    onesd = nc.dram_tensor("ones", [128, 1], F32, kind="ExternalInput").ap()
    cbiasd = nc.dram_tensor("cbias", [128, 2], F32, kind="ExternalInput").ap()
    batchd = nc.dram_tensor("batchf", [128, HI], F32, kind="ExternalInput").ap()
    outd = nc.dram_tensor("out", [1, 1], F32, kind="ExternalOutput").ap()

    def col(t, field, i):
        c = field * T + i
        return t[:, c:c + 1]

    with tile.TileContext(nc) as tc:
        with (
            tc.tile_pool(name="const", bufs=1) as cpool,
            tc.tile_pool(name="edge", bufs=1) as epool,
            tc.tile_pool(name="work", bufs=3) as wpool,
            tc.tile_pool(name="small", bufs=1) as spool,
            tc.tile_pool(name="psum", bufs=1, space="PSUM") as ppool,
            tc.tile_pool(name="dram", bufs=1, space="DRAM") as dpool,
        ):
            io128 = cpool.tile([128, LO], F32, tag="io128")
            nc.sync.dma_start(io128[:], iota128)
            io32 = cpool.tile([128, HI], F32, tag="io32")
            nc.sync.dma_start(io32[:], iota32)
            ones_t = cpool.tile([128, 1], F32, tag="ones")
            nc.sync.dma_start(ones_t[:], onesd)
            cb = cpool.tile([128, 2], F32, tag="cb")
            nc.sync.dma_start(cb[:], cbiasd)
            bzero = cb[:, 0:1]
            bt = cpool.tile([128, HI], F32, tag="bt")
            nc.sync.dma_start(bt[:], batchd)
            ed = epool.tile([128, 7 * T], F32, tag="ed")
            nc.sync.dma_start(ed[:], edata)

            tf = ed[:, _F_TF * T:(_F_TF + 1) * T]
            uf = ed[:, _F_UF * T:(_F_UF + 1) * T]
            pp = ed[:, _F_P * T:(_F_P + 1) * T]

            # batched per-edge prep (one op per quantity, all T columns at once)
            logmsg = spool.tile([128, T], F32, tag="logmsg")
            nc.scalar.activation(logmsg[:], pp, ACT.Ln, scale=-1.0, bias=cb[:, 1:2])
            m = spool.tile([128, T], F32, tag="m")
            nc.vector.tensor_tensor(m[:], tf, uf, op=ALU.is_equal)
            valu = spool.tile([128, T], F32, tag="valu")  # p * (1 - m)
            nc.vector.scalar_tensor_tensor(
                valu[:], m[:], 0.5, pp, op0=ALU.is_lt, op1=ALU.mult
            )
            sq = spool.tile([128, T], F32, tag="sq")
            nc.scalar.activation(sq[:], pp, ACT.Square, bias=bzero)
            sqm = spool.tile([128, T], F32, tag="sqm")
            nc.vector.tensor_tensor(sqm[:], sq[:], m[:], op=ALU.mult)

            C = spool.tile([128, 65], F32, tag="C")  # [log_score | s | dp2]
            dp2scr = spool.tile([128, T], F32, tag="dp2scr")
            # dp2 = 2*sq - sq*m summed over free dim straight into C[:,64]
            nc.vector.scalar_tensor_tensor(
                dp2scr[:], sq[:], 2.0, sqm[:],
                op0=ALU.mult, op1=ALU.subtract, accum_out=C[:, 64:65],
            )

            P1 = ppool.tile([128, HI], F32, tag="P1")  # log_score [lo, hi]
            P2 = ppool.tile([128, HI], F32, tag="P2")  # s         [lo, hi]
            for i in range(T):
                At = wpool.tile([128, LO], F32, tag="At")
                nc.vector.tensor_scalar(
                    At[:], io128[:], col(ed, _F_TLO, i), None, op0=ALU.is_equal
                )
                Au = wpool.tile([128, LO], F32, tag="Au")
                nc.gpsimd.tensor_scalar(
                    Au[:], io128[:], col(ed, _F_ULO, i), None, op0=ALU.is_equal
                )
                rp = wpool.tile([128, HI], F32, tag="rp")
                nc.vector.tensor_scalar(
                    rp[:], io32[:], col(ed, _F_THI, i), logmsg[:, i:i + 1],
                    op0=ALU.is_equal, op1=ALU.mult,
                )
                rst = wpool.tile([128, HI], F32, tag="rst")
                nc.vector.tensor_scalar(
                    rst[:], io32[:], col(ed, _F_THI, i), pp[:, i:i + 1],
                    op0=ALU.is_equal, op1=ALU.mult,
                )
                rsu = wpool.tile([128, HI], F32, tag="rsu")
                nc.vector.tensor_scalar(
                    rsu[:], io32[:], col(ed, _F_UHI, i), valu[:, i:i + 1],
                    op0=ALU.is_equal, op1=ALU.mult,
                )
                nc.tensor.matmul(P1[:], At[:], rp[:], start=(i == 0), stop=(i == T - 1))
                nc.tensor.matmul(P2[:], At[:], rst[:], start=(i == 0), stop=False)
                nc.tensor.matmul(P2[:], Au[:], rsu[:], start=False, stop=(i == T - 1))

            nc.scalar.copy(C[:, 0:32], P1[:])
            nc.vector.tensor_copy(C[:, 32:64], P2[:])

            if use_collective:
                cin = dpool.tile([128, 65], F32, tag="cin")
                cout = dpool.tile([128, 65], F32, tag="cout", addr_space="Shared")
                nc.sync.dma_start(cin[:], C[:])
                nc.gpsimd.collective_compute(
                    "AllReduce",
                    ALU.add,
                    replica_groups=[list(range(N_CORES))],
                    ins=[cin.opt()],
                    outs=[cout.opt()],
                )
                C2 = spool.tile([128, 65], F32, tag="C2")
                nc.sync.dma_start(C2[:], cout[:])
            else:
                C2 = C

            # final reduction (every core, redundant)
            R = spool.tile([128, 3], F32, tag="R")
            scr1 = spool.tile([128, HI], F32, tag="scr1")
            nc.scalar.activation(scr1[:], C2[:, 0:32], ACT.Exp, bias=bzero, accum_out=R[:, 0:1])
            scr2 = spool.tile([128, HI], F32, tag="scr2")
            nc.scalar.activation(scr2[:], C2[:, 32:64], ACT.Square, bias=bzero, accum_out=R[:, 1:2])
            nc.vector.tensor_copy(R[:, 2:3], C2[:, 64:65])

            F = ppool.tile([1, 3], F32, tag="F")
            nc.tensor.matmul(F[:], ones_t[:], R[:], start=True, stop=True)
            Fs = spool.tile([1, 3], F32, tag="Fs")
            nc.scalar.copy(Fs[:], F[:])

            # num_graphs = max(batch) + 1, on device
            bmax = spool.tile([128, 1], F32, tag="bmax")
            nc.vector.tensor_reduce(bmax[:], bt[:], axis=mybir.AxisListType.X, op=ALU.max)
            ball = spool.tile([128, 1], F32, tag="ball")
            nc.gpsimd.partition_all_reduce(
                ball[:], bmax[:], channels=128, reduce_op=bass_isa.ReduceOp.max
            )
            ng = spool.tile([1, 1], F32, tag="ng")
            nc.vector.tensor_scalar_add(ng[:], ball[0:1, 0:1], 1.0)
            rng = spool.tile([1, 1], F32, tag="rng")
            nc.vector.reciprocal(rng[:], ng[:])

            # loss2 = (sumL - PAD_NODES) * SCALE/N ; t2 = (sumS2-sumDP2)*100/ng
            l2 = spool.tile([1, 1], F32, tag="l2")
            nc.vector.tensor_scalar(
                l2[:], Fs[:, 0:1], -float(PAD_NODES), PENALTY_SCALE / N_NODES,
                op0=ALU.add, op1=ALU.mult,
            )
            d32 = spool.tile([1, 1], F32, tag="d32")
            nc.vector.tensor_tensor(d32[:], Fs[:, 1:2], Fs[:, 2:3], op=ALU.subtract)
            t2 = spool.tile([1, 1], F32, tag="t2")
            nc.vector.scalar_tensor_tensor(
                t2[:], d32[:], 100.0, rng[:], op0=ALU.mult, op1=ALU.mult
            )
            res = spool.tile([1, 1], F32, tag="res")
            nc.vector.tensor_tensor(res[:], l2[:], t2[:], op=ALU.add)
            nc.sync.dma_start(outd, res[:])

    nc.compile()
    return nc


def _pack_core(tt, uu, p, n_tiles):
    """Pack one core's edge shard into the [128, 7*T] fp32 edata layout."""
    T = n_tiles
    ne = tt.shape[0]
    npad = T * 128

    def pad(a, fill):
        out = np.full(npad, fill, np.float64)
        out[:ne] = a
        return out.reshape(T, 128).T.astype(np.float32)  # [128, T]

    t_lo = pad(tt % 128, 0.0)
    t_hi = pad(tt // 128, float(HI))     # sentinel hi -> matches nothing
    u_lo = pad(uu % 128, 0.0)
    u_hi = pad(uu // 128, float(HI))
    tf = pad(tt, 0.0)
    uf = pad(uu, 0.0)                    # pad: tf==uf -> m=1, but p=0
    pf = pad(p, 0.0)
    return np.concatenate([t_lo, t_hi, u_lo, u_hi, tf, uf, pf], axis=1)


_CACHE = {}


def _get_program(tiles_per_core, use_collective):
    key = (tiles_per_core, use_collective)
    if key not in _CACHE:
        _CACHE[key] = _build_program(tiles_per_core, use_collective)
    return _CACHE[key]


def kernel(x, edge_index, edge_feature, batch, _trace=False):
    x = np.asarray(x)
    ei = np.asarray(edge_index).astype(np.int64)
    p = np.asarray(edge_feature).astype(np.float32)[:, 0]
    batch = np.asarray(batch).astype(np.int64)

    uu_all = ei[0].astype(np.float64)
    tt_all = ei[1].astype(np.float64)

    iota128 = np.tile(np.arange(LO, dtype=np.float32), (128, 1))
    iota32 = np.tile(np.arange(HI, dtype=np.float32), (128, 1))
    ones = np.ones((128, 1), np.float32)
    cbias = np.zeros((128, 2), np.float32)
    cbias[:, 1] = 1.0 + 1e-6
    bpad = np.zeros(N_PAD, np.float32)
    bpad[:N_NODES] = batch.astype(np.float32)
    batchf = bpad.reshape(128, HI)

    nc = _get_program(TPC, True)
    in_maps = []
    for c in range(N_CORES):
        sl = slice(c * EPC, (c + 1) * EPC)
        edata = _pack_core(tt_all[sl], uu_all[sl], p[sl], TPC)
        in_maps.append({
            "edata": edata,
            "iota128": iota128,
            "iota32": iota32,
            "ones": ones,
            "cbias": cbias,
            "batchf": batchf,
        })

    r = bass_utils.run_bass_kernel_spmd(
        nc, in_maps, core_ids=list(range(N_CORES)), trace=_trace
    )
    out = np.asarray(r.results[0]["out"], dtype=np.float32).reshape(1, 1)
    if _trace:
        kernel.last_result = r
    return out


# revision 4
# speedup vs baseline: 2.4550x; 2.4550x over previous
"""Trainium2 Bass kernel for nn_ErdosLoss (graph loss function).

Math (reference reformulated, validated to ~1e-6 rel err):
  penalty:  log_score = scatter_add(log(1 - p + 1e-6), tgt)   over N nodes
            loss2 = mean(exp(log_score)) * 9600
  loss3:    p @ triu(H H^T, 1) @ p^T  ==  (||s||^2 - sum_e d_e p_e^2) / 2
            where s = scatter_add(p, tgt) + scatter_add(p * (1-m), src),
            m_e = (src_e == tgt_e)  (H rows are node *sets*: self-loops get a
            single 1), d_e = 2 - m_e.
  out = loss2 + 200 * loss3 / num_graphs,  num_graphs = max(batch) + 1.

Device strategy (8 NeuronCores, SPMD, two launches, no collectives):
  Launch skew across the 8 PJRT devices is ~30us here, so any cross-core
  barrier (AllReduce) inflates every core's exec time by the skew plus a
  ~13us collective.  Instead:
  - Phase 1 (8 cores, edge-sharded 750/core): scatter-add via one-hot
    matmul with node = 128*hi + lo decomposition (N padded to 4096).
    One-hots are built as a handful of *wide* DVE ops using stride-0
    broadcast APs (per-instruction overhead dominates at this size), cast
    to f16 (exact for 0/1 and for values' 10-bit precision needs), and
    contracted on TensorE into PSUM [128lo, 64] (= log_score | s).
    Each core writes a [128, 65] partial (log_score | s | dp2 rowsum).
  - Host gathers the 8 partials (pure data movement) into [128, 520].
  - Phase 2 (1 core): tree-add the 8 partials, exp/square row-sums,
    ones-matmul partition reduce, max(batch)+1 on device, final scalar.
"""

import os
import numpy as np

import concourse.bass as bass
import concourse.bacc as bacc
import concourse.mybir as mybir
import concourse.tile as tile
import concourse.bass_isa as bass_isa
from concourse import bass_utils

F32 = mybir.dt.float32
F16 = mybir.dt.float16
ALU = mybir.AluOpType
ACT = mybir.ActivationFunctionType
AX = mybir.AxisListType

N_NODES = 4000
N_EDGES = 6000
N_CORES = 8
N_PAD = 4096          # 128 * 32
HI = 32               # node hi-digits
LO = 128              # node lo-digits
PENALTY_SCALE = 16 * 200 * 3   # 9600
PAD_NODES = N_PAD - N_NODES    # 96 padded nodes, each contributes exp(0)=1

EPC = N_EDGES // N_CORES       # 750 edges per core
TPC = (EPC + 127) // 128       # 6 edge tiles per core

# edata column layout: 7 fields x T columns (lo pair and hi pair adjacent)
_F_TLO, _F_ULO, _F_THI, _F_UHI, _F_TF, _F_UF, _F_P = range(7)


def _common_inputs(nc):
    iota128 = nc.dram_tensor("iota128", [128, LO], F32, kind="ExternalInput").ap()
    iota32 = nc.dram_tensor("iota32", [128, HI], F32, kind="ExternalInput").ap()
    return iota128, iota32


def _build_phase1(T: int):
    """Per-core partial computation: out 'partial' [128, 65]."""
    nc = bacc.Bacc("TRN2", target_bir_lowering=False, debug=False, num_devices=1)

    edata = nc.dram_tensor("edata", [128, 7 * T], F32, kind="ExternalInput").ap()
    iota128, iota32 = _common_inputs(nc)
    cbiasd = nc.dram_tensor("cbias", [128, 2], F32, kind="ExternalInput").ap()
    partiald = nc.dram_tensor("partial", [128, 65], F32, kind="ExternalOutput").ap()

    with tile.TileContext(nc) as tc:
        with (
            tc.tile_pool(name="const", bufs=1) as cpool,
            tc.tile_pool(name="work", bufs=1) as wpool,
            tc.tile_pool(name="psum", bufs=1, space="PSUM") as ppool,
        ):
            io128 = cpool.tile([128, LO], F32, tag="io128")
            nc.sync.dma_start(io128[:], iota128)
            io32 = cpool.tile([128, HI], F32, tag="io32")
            nc.sync.dma_start(io32[:], iota32)
            cb = cpool.tile([128, 2], F32, tag="cb")
            nc.sync.dma_start(cb[:], cbiasd)
            ed = cpool.tile([128, 7 * T], F32, tag="ed")
            nc.sync.dma_start(ed[:], edata)

            lo_pair = ed[:, 0:2 * T]                       # [tlo | ulo]
            hi_pair = ed[:, 2 * T:4 * T]                   # [thi | uhi]
            tf = ed[:, _F_TF * T:(_F_TF + 1) * T]
            uf = ed[:, _F_UF * T:(_F_UF + 1) * T]
            pp = ed[:, _F_P * T:(_F_P + 1) * T]

            C = wpool.tile([128, 65], F32, tag="C")

            # ---- small per-edge prep, batched over all T columns
            m = wpool.tile([128, T], F32, tag="m")
            nc.vector.tensor_tensor(m[:], tf, uf, op=ALU.is_equal)
            valu = wpool.tile([128, T], F32, tag="valu")   # p * (1 - m)
            nc.vector.scalar_tensor_tensor(
                valu[:], m[:], 0.5, pp, op0=ALU.is_lt, op1=ALU.mult
            )
            # dp2 = p^2 (2 - m) = (valu + p) * p, row-summed into C[:,64]
            tsum = wpool.tile([128, T], F32, tag="tsum")
            nc.vector.tensor_tensor(tsum[:], valu[:], pp, op=ALU.add)
            dp2scr = wpool.tile([128, T], F32, tag="dp2scr")
            nc.vector.scalar_tensor_tensor(
                dp2scr[:], tsum[:], 1.0, pp,
                op0=ALU.mult, op1=ALU.mult, accum_out=C[:, 64:65],
            )
            # V = [logmsg | p] for the rp/rst build
            V = wpool.tile([128, 2 * T], F32, tag="V")
            nc.scalar.activation(V[:, 0:T], pp, ACT.Ln, scale=-1.0, bias=cb[:, 1:2])
            nc.scalar.copy(V[:, T:2 * T], pp)

            # ---- one-hots (f16, exact), few wide ops via stride-0 APs
            A_all = wpool.tile([128, 2 * T * LO], F16, tag="A_all")
            nc.vector.tensor_tensor(
                A_all[:].rearrange("p (t l) -> p t l", l=LO),
                io128[:].rearrange("p (o l) -> p o l", o=1).to_broadcast((128, 2 * T, LO)),
                lo_pair.rearrange("p (t o) -> p t o", o=1).to_broadcast((128, 2 * T, LO)),
                op=ALU.is_equal,
            )
            H_all = wpool.tile([128, 2 * T * HI], F16, tag="H_all")
            nc.vector.tensor_tensor(
                H_all[:].rearrange("p (t h) -> p t h", h=HI),
                io32[:].rearrange("p (o h) -> p o h", o=1).to_broadcast((128, 2 * T, HI)),
                hi_pair.rearrange("p (t o) -> p t o", o=1).to_broadcast((128, 2 * T, HI)),
                op=ALU.is_equal,
            )
            # RS_all: per tile i the contiguous [rp_i(32) | rst_i(32)]
            RS_all = wpool.tile([128, T * 64], F16, tag="RS_all")
            nc.vector.tensor_tensor(
                RS_all[:].rearrange("p (t o h) -> p o t h", o=2, h=HI),
                H_all[:, 0:T * HI].rearrange("p (o t h) -> p o t h", o=1, h=HI)
                    .to_broadcast((128, 2, T, HI)),
                V[:].rearrange("p (o t) -> p o t", o=2)
                    .rearrange("p o (t h) -> p o t h", h=1)
                    .to_broadcast((128, 2, T, HI)),
                op=ALU.mult,
            )
            rsu_all = wpool.tile([128, T * HI], F16, tag="rsu_all")
            nc.vector.tensor_tensor(
                rsu_all[:].rearrange("p (t h) -> p t h", h=HI),
                H_all[:, T * HI:2 * T * HI].rearrange("p (t h) -> p t h", h=HI),
                valu[:].rearrange("p (t o) -> p t o", o=1).to_broadcast((128, T, HI)),
                op=ALU.mult,
            )

            # ---- scatter-add matmuls: P12 = [log_score(32) | s(32)]
            P12 = ppool.tile([128, 64], F32, tag="P12")
            for i in range(T):
                nc.tensor.matmul(
                    P12[:, 0:64],
                    A_all[:, i * LO:(i + 1) * LO],
                    RS_all[:, i * 64:(i + 1) * 64],
                    start=(i == 0), stop=False, skip_group_check=True,
                )
                nc.tensor.matmul(
                    P12[:, 32:64],
                    A_all[:, (T + i) * LO:(T + i + 1) * LO],
                    rsu_all[:, i * HI:(i + 1) * HI],
                    start=False, stop=(i == T - 1), skip_group_check=True,
                )

            nc.scalar.copy(C[:, 0:64], P12[:])
            nc.sync.dma_start(partiald, C[:])

    nc.compile()
    return nc


def _build_phase2():
    """Combine 8 partials -> final scalar. Runs on one core."""
    nc = bacc.Bacc("TRN2", target_bir_lowering=False, debug=False, num_devices=1)

    partsd = nc.dram_tensor("parts", [128, 8 * 65], F32, kind="ExternalInput").ap()
    onesd = nc.dram_tensor("ones", [128, 1], F32, kind="ExternalInput").ap()
    cbiasd = nc.dram_tensor("cbias", [128, 2], F32, kind="ExternalInput").ap()
    batchd = nc.dram_tensor("batchf", [128, HI], F32, kind="ExternalInput").ap()
    outd = nc.dram_tensor("out", [1, 1], F32, kind="ExternalOutput").ap()

    with tile.TileContext(nc) as tc:
        with (
            tc.tile_pool(name="pool", bufs=1) as pool,
            tc.tile_pool(name="psum", bufs=1, space="PSUM") as ppool,
        ):
            pt = pool.tile([128, 8 * 65], F32, tag="pt")
            nc.sync.dma_start(pt[:], partsd)
            ones_t = pool.tile([128, 1], F32, tag="ones")
            nc.sync.dma_start(ones_t[:], onesd)
            cb = pool.tile([128, 2], F32, tag="cb")
            nc.sync.dma_start(cb[:], cbiasd)
            bt = pool.tile([128, HI], F32, tag="bt")
            nc.sync.dma_start(bt[:], batchd)
            bzero = cb[:, 0:1]

            t1 = pool.tile([128, 4 * 65], F32, tag="t1")
            nc.vector.tensor_tensor(t1[:], pt[:, 0:260], pt[:, 260:520], op=ALU.add)
            t2 = pool.tile([128, 2 * 65], F32, tag="t2")
            nc.vector.tensor_tensor(t2[:], t1[:, 0:130], t1[:, 130:260], op=ALU.add)
            C2 = pool.tile([128, 65], F32, tag="C2")
            nc.vector.tensor_tensor(C2[:], t2[:, 0:65], t2[:, 65:130], op=ALU.add)

            R = pool.tile([128, 3], F32, tag="R")
            scr1 = pool.tile([128, HI], F32, tag="scr1")
            nc.scalar.activation(scr1[:], C2[:, 0:32], ACT.Exp, bias=bzero,
                                 accum_out=R[:, 0:1])
            scr2 = pool.tile([128, HI], F32, tag="scr2")
            nc.vector.scalar_tensor_tensor(
                scr2[:], C2[:, 32:64], 1.0, C2[:, 32:64],
                op0=ALU.mult, op1=ALU.mult, accum_out=R[:, 1:2],
            )
            nc.vector.tensor_copy(R[:, 2:3], C2[:, 64:65])

            F = ppool.tile([1, 3], F32, tag="F")
            nc.tensor.matmul(F[:], ones_t[:], R[:], start=True, stop=True)
            Fs = pool.tile([1, 3], F32, tag="Fs")
            nc.scalar.copy(Fs[:], F[:])

            # num_graphs = max(batch) + 1, on device
            bmax = pool.tile([128, 1], F32, tag="bmax")
            nc.vector.tensor_reduce(bmax[:], bt[:], axis=AX.X, op=ALU.max)
            ball = pool.tile([128, 1], F32, tag="ball")
            nc.gpsimd.partition_all_reduce(
                ball[:], bmax[:], channels=128, reduce_op=bass_isa.ReduceOp.max
            )
            ng = pool.tile([1, 1], F32, tag="ng")
            nc.vector.tensor_scalar_add(ng[:], ball[0:1, 0:1], 1.0)
            rng = pool.tile([1, 1], F32, tag="rng")
            nc.vector.reciprocal(rng[:], ng[:])

            l2 = pool.tile([1, 1], F32, tag="l2")
            nc.vector.tensor_scalar(
                l2[:], Fs[:, 0:1], -float(PAD_NODES), PENALTY_SCALE / N_NODES,
                op0=ALU.add, op1=ALU.mult,
            )
            d32 = pool.tile([1, 1], F32, tag="d32")
            nc.vector.tensor_tensor(d32[:], Fs[:, 1:2], Fs[:, 2:3], op=ALU.subtract)
            t2s = pool.tile([1, 1], F32, tag="t2s")
            nc.vector.scalar_tensor_tensor(
                t2s[:], d32[:], 100.0, rng[:], op0=ALU.mult, op1=ALU.mult
            )
            res = pool.tile([1, 1], F32, tag="res")
            nc.vector.tensor_tensor(res[:], l2[:], t2s[:], op=ALU.add)
            nc.sync.dma_start(outd, res[:])

    nc.compile()
    return nc


def _pack_core(tt, uu, p, T):
    """Pack one core's edge shard into the [128, 7*T] fp32 edata layout."""
    ne = tt.shape[0]
    npad = T * 128

    def pad(a, fill):
        out = np.full(npad, fill, np.float64)
        out[:ne] = a
        return out.reshape(T, 128).T.astype(np.float32)  # [128, T]

    t_lo = pad(tt % 128, 0.0)
    t_hi = pad(tt // 128, float(HI))     # sentinel hi -> matches nothing
    u_lo = pad(uu % 128, 0.0)
    u_hi = pad(uu // 128, float(HI))
    tf = pad(tt, 0.0)
    uf = pad(uu, 0.0)                    # pad: tf==uf -> m=1, but p=0
    pf = pad(p, 0.0)
    return np.concatenate([t_lo, u_lo, t_hi, u_hi, tf, uf, pf], axis=1)


_CACHE = {}


def _get(name, builder, *a):
    if name not in _CACHE:
        _CACHE[name] = builder(*a)
    return _CACHE[name]


def kernel(x, edge_index, edge_feature, batch, _trace=False):
    x = np.asarray(x)
    ei = np.asarray(edge_index).astype(np.int64)
    p = np.asarray(edge_feature).astype(np.float32)[:, 0]
    batch = np.asarray(batch).astype(np.int64)

    uu_all = ei[0].astype(np.float64)
    tt_all = ei[1].astype(np.float64)

    iota128 = np.tile(np.arange(LO, dtype=np.float32), (128, 1))
    iota32 = np.tile(np.arange(HI, dtype=np.float32), (128, 1))
    ones = np.ones((128, 1), np.float32)
    cbias = np.zeros((128, 2), np.float32)
    cbias[:, 1] = 1.0 + 1e-6
    bpad = np.zeros(N_PAD, np.float32)
    bpad[:N_NODES] = batch.astype(np.float32)
    batchf = bpad.reshape(128, HI)

    # ---- phase 1: per-core partials (no cross-core dependencies)
    nc1 = _get("p1", _build_phase1, TPC)
    in_maps = []
    for c in range(N_CORES):
        sl = slice(c * EPC, (c + 1) * EPC)
        in_maps.append({
            "edata": _pack_core(tt_all[sl], uu_all[sl], p[sl], TPC),
            "iota128": iota128,
            "iota32": iota32,
            "cbias": cbias,
        })
    r1 = bass_utils.run_bass_kernel_spmd(
        nc1, in_maps, core_ids=list(range(N_CORES)), trace=_trace
    )

    # gather/unshard the per-core partials (pure data movement)
    parts = np.concatenate(
        [np.asarray(r1.results[c]["partial"], np.float32) for c in range(N_CORES)],
        axis=1,
    )

    # ---- phase 2: combine on one core
    nc2 = _get("p2", _build_phase2)
    r2 = bass_utils.run_bass_kernel_spmd(
        nc2,
        [{"parts": parts, "ones": ones, "cbias": cbias, "batchf": batchf}],
        core_ids=[0],
        trace=_trace,
    )
    out = np.asarray(r2.results[0]["out"], dtype=np.float32).reshape(1, 1)
    if _trace:
        kernel.last_results = (r1, r2)
    return out


# revision 5
# speedup vs baseline: 2.5652x; 1.0449x over previous
"""Trainium2 Bass kernel for nn_ErdosLoss (graph loss function).

Math (reference reformulated, validated to ~1e-6 rel err):
  penalty:  log_score = scatter_add(log(1 - p + 1e-6), tgt)   over N nodes
            loss2 = mean(exp(log_score)) * 9600
  loss3:    p @ triu(H H^T, 1) @ p^T  ==  (||s||^2 - sum_e d_e p_e^2) / 2
            where s = scatter_add(p, tgt) + scatter_add(p * (1-m), src),
            m_e = (src_e == tgt_e)  (H rows are node *sets*: self-loops get a
            single 1), d_e = 2 - m_e.
  out = loss2 + 200 * loss3 / num_graphs,  num_graphs = max(batch) + 1.

Device strategy (8 NeuronCores, SPMD, two launches, no collectives):
  Launch skew across the 8 PJRT devices is ~30us here, so any cross-core
  barrier (AllReduce) inflates every core's exec time by the skew plus a
  ~13us collective.  Instead:
  - Phase 1 (8 cores, edge-sharded 750/core): scatter-add via one-hot
    matmul with node = 128*hi + lo decomposition (N padded to 4096).
    One-hots are built as a handful of *wide* DVE ops using stride-0
    broadcast APs (per-instruction overhead dominates at this size), cast
    to f16 (exact for 0/1 and for values' 10-bit precision needs), and
    contracted on TensorE into PSUM [128lo, 64] (= log_score | s).
    Each core writes a [128, 65] partial (log_score | s | dp2 rowsum).
  - Host gathers the 8 partials (pure data movement) into [128, 520].
  - Phase 2 (1 core): tree-add the 8 partials, exp/square row-sums,
    ones-matmul partition reduce, max(batch)+1 on device, final scalar.
"""

import os
import numpy as np

import concourse.bass as bass
import concourse.bacc as bacc
import concourse.mybir as mybir
import concourse.tile as tile
import concourse.bass_isa as bass_isa
from concourse import bass_utils

F32 = mybir.dt.float32
F16 = mybir.dt.float16
ALU = mybir.AluOpType
ACT = mybir.ActivationFunctionType
AX = mybir.AxisListType

N_NODES = 4000
N_EDGES = 6000
N_CORES = 8
N_PAD = 4096          # 128 * 32
HI = 32               # node hi-digits
LO = 128              # node lo-digits
PENALTY_SCALE = 16 * 200 * 3   # 9600
PAD_NODES = N_PAD - N_NODES    # 96 padded nodes, each contributes exp(0)=1

EPC = N_EDGES // N_CORES       # 750 edges per core
TPC = (EPC + 127) // 128       # 6 edge tiles per core

# edata column layout: 7 fields x T columns (lo pair and hi pair adjacent)
_F_TLO, _F_ULO, _F_THI, _F_UHI, _F_TF, _F_UF, _F_P = range(7)


def _common_inputs(nc):
    iota128 = nc.dram_tensor("iota128", [128, LO], F32, kind="ExternalInput").ap()
    iota32 = nc.dram_tensor("iota32", [128, HI], F32, kind="ExternalInput").ap()
    return iota128, iota32


def _build_phase1(T: int):
    """Per-core partial computation: out 'partial' [128, 65]."""
    nc = bacc.Bacc("TRN2", target_bir_lowering=False, debug=False, num_devices=1)

    # single input blob: [iota128(128) | iota32(32) | cbias(2) | edata(7T)]
    BW = LO + HI + 2 + 7 * T
    blobd = nc.dram_tensor("blob", [128, BW], F32, kind="ExternalInput").ap()
    partiald = nc.dram_tensor("partial", [128, 65], F32, kind="ExternalOutput").ap()
    EO = LO + HI + 2   # edata offset inside blob

    with tile.TileContext(nc) as tc:
        with (
            tc.tile_pool(name="const", bufs=1) as cpool,
            tc.tile_pool(name="work", bufs=1) as wpool,
            tc.tile_pool(name="psum", bufs=1, space="PSUM") as ppool,
        ):
            # warm the Ln ACT table while the input DMA is in flight
            wz = cpool.tile([128, 1], F32, tag="wz")
            nc.vector.memset(wz[:], 0.5)
            wb = cpool.tile([128, 1], F32, tag="wb")
            nc.gpsimd.memset(wb[:], 0.0)
            wo = cpool.tile([128, 1], F32, tag="wo")
            nc.scalar.activation(wo[:], wz[:], ACT.Ln, bias=wb[:])

            blob = cpool.tile([128, BW], F32, tag="blob")
            nc.sync.dma_start(blob[:], blobd)
            io128 = blob[:, 0:LO]
            io32 = blob[:, LO:LO + HI]
            cb = blob[:, LO + HI:LO + HI + 2]
            ed = blob[:, EO:EO + 7 * T]

            lo_pair = ed[:, 0:2 * T]                       # [tlo | ulo]
            hi_pair = ed[:, 2 * T:4 * T]                   # [thi | uhi]
            tf = ed[:, _F_TF * T:(_F_TF + 1) * T]
            uf = ed[:, _F_UF * T:(_F_UF + 1) * T]
            pp = ed[:, _F_P * T:(_F_P + 1) * T]

            C = wpool.tile([128, 65], F32, tag="C")

            # ---- one-hots (f16, exact), few wide ops via stride-0 APs
            A_all = wpool.tile([128, 2 * T * LO], F16, tag="A_all")
            nc.vector.tensor_tensor(
                A_all[:].rearrange("p (t l) -> p t l", l=LO),
                io128.rearrange("p (o l) -> p o l", o=1).to_broadcast((128, 2 * T, LO)),
                lo_pair.rearrange("p (t o) -> p t o", o=1).to_broadcast((128, 2 * T, LO)),
                op=ALU.is_equal,
            )
            H_all = wpool.tile([128, 2 * T * HI], F16, tag="H_all")
            nc.vector.tensor_tensor(
                H_all[:].rearrange("p (t h) -> p t h", h=HI),
                io32.rearrange("p (o h) -> p o h", o=1).to_broadcast((128, 2 * T, HI)),
                hi_pair.rearrange("p (t o) -> p t o", o=1).to_broadcast((128, 2 * T, HI)),
                op=ALU.is_equal,
            )
            # V = [logmsg | p] on the ACT engine (parallel to the DVE ops)
            V = wpool.tile([128, 2 * T], F32, tag="V")
            nc.scalar.activation(V[:, 0:T], pp, ACT.Ln, scale=-1.0, bias=cb[:, 1:2])
            nc.scalar.copy(V[:, T:2 * T], pp)

            # RS_all: per tile i the contiguous [rp_i(32) | rst_i(32)]
            RS_all = wpool.tile([128, T * 64], F16, tag="RS_all")
            nc.vector.tensor_tensor(
                RS_all[:].rearrange("p (t o h) -> p o t h", o=2, h=HI),
                H_all[:, 0:T * HI].rearrange("p (o t h) -> p o t h", o=1, h=HI)
                    .to_broadcast((128, 2, T, HI)),
                V[:].rearrange("p (o t) -> p o t", o=2)
                    .rearrange("p o (t h) -> p o t h", h=1)
                    .to_broadcast((128, 2, T, HI)),
                op=ALU.mult,
            )
            # small per-edge prep
            m = wpool.tile([128, T], F32, tag="m")
            nc.vector.tensor_tensor(m[:], tf, uf, op=ALU.is_equal)
            valu = wpool.tile([128, T], F32, tag="valu")   # p * (1 - m)
            nc.vector.scalar_tensor_tensor(
                valu[:], m[:], 0.5, pp, op0=ALU.is_lt, op1=ALU.mult
            )
            rsu_all = wpool.tile([128, T * HI], F16, tag="rsu_all")
            nc.vector.tensor_tensor(
                rsu_all[:].rearrange("p (t h) -> p t h", h=HI),
                H_all[:, T * HI:2 * T * HI].rearrange("p (t h) -> p t h", h=HI),
                valu[:].rearrange("p (t o) -> p t o", o=1).to_broadcast((128, T, HI)),
                op=ALU.mult,
            )
            # dp2 = p^2 (2 - m) = (valu + p) * p, row-summed into C[:,64]
            tsum = wpool.tile([128, T], F32, tag="tsum")
            nc.vector.tensor_tensor(tsum[:], valu[:], pp, op=ALU.add)
            dp2scr = wpool.tile([128, T], F32, tag="dp2scr")
            nc.vector.scalar_tensor_tensor(
                dp2scr[:], tsum[:], 1.0, pp,
                op0=ALU.mult, op1=ALU.mult, accum_out=C[:, 64:65],
            )

            # ---- scatter-add matmuls: P12 = [log_score(32) | s(32)]
            P12 = ppool.tile([128, 64], F32, tag="P12")
            for i in range(T):
                nc.tensor.matmul(
                    P12[:, 0:64],
                    A_all[:, i * LO:(i + 1) * LO],
                    RS_all[:, i * 64:(i + 1) * 64],
                    start=(i == 0), stop=False, skip_group_check=True,
                )
            for i in range(T):
                nc.tensor.matmul(
                    P12[:, 32:64],
                    A_all[:, (T + i) * LO:(T + i + 1) * LO],
                    rsu_all[:, i * HI:(i + 1) * HI],
                    start=False, stop=(i == T - 1), skip_group_check=True,
                )

            nc.scalar.copy(C[:, 0:64], P12[:])
            nc.sync.dma_start(partiald, C[:])

    nc.compile()
    return nc


def _build_phase2():
    """Combine 8 partials -> final scalar. Runs on one core."""
    nc = bacc.Bacc("TRN2", target_bir_lowering=False, debug=False, num_devices=1)

    # blob: [parts(520) | ones(1) | cbias(2) | batchf(32)]
    BW = 520 + 1 + 2 + HI
    blobd = nc.dram_tensor("blob2", [128, BW], F32, kind="ExternalInput").ap()
    outd = nc.dram_tensor("out", [1, 1], F32, kind="ExternalOutput").ap()

    with tile.TileContext(nc) as tc:
        with (
            tc.tile_pool(name="pool", bufs=1) as pool,
            tc.tile_pool(name="psum", bufs=1, space="PSUM") as ppool,
        ):
            wz = pool.tile([128, 1], F32, tag="wz")
            nc.vector.memset(wz[:], 0.5)
            wb = pool.tile([128, 1], F32, tag="wb")
            nc.gpsimd.memset(wb[:], 0.0)
            wo = pool.tile([128, 1], F32, tag="wo")
            nc.scalar.activation(wo[:], wz[:], ACT.Exp, bias=wb[:])

            blob = pool.tile([128, BW], F32, tag="blob")
            nc.sync.dma_start(blob[:], blobd)
            pt = blob[:, 0:520]
            ones_t = blob[:, 520:521]
            cb = blob[:, 521:523]
            bt = blob[:, 523:523 + HI]
            bzero = cb[:, 0:1]

            t1 = pool.tile([128, 4 * 65], F32, tag="t1")
            nc.vector.tensor_tensor(t1[:], pt[:, 0:260], pt[:, 260:520], op=ALU.add)
            t2 = pool.tile([128, 2 * 65], F32, tag="t2")
            nc.vector.tensor_tensor(t2[:], t1[:, 0:130], t1[:, 130:260], op=ALU.add)
            C2 = pool.tile([128, 65], F32, tag="C2")
            nc.vector.tensor_tensor(C2[:], t2[:, 0:65], t2[:, 65:130], op=ALU.add)

            R = pool.tile([128, 3], F32, tag="R")
            scr1 = pool.tile([128, HI], F32, tag="scr1")
            nc.scalar.activation(scr1[:], C2[:, 0:32], ACT.Exp, bias=bzero,
                                 accum_out=R[:, 0:1])
            scr2 = pool.tile([128, HI], F32, tag="scr2")
            nc.vector.scalar_tensor_tensor(
                scr2[:], C2[:, 32:64], 1.0, C2[:, 32:64],
                op0=ALU.mult, op1=ALU.mult, accum_out=R[:, 1:2],
            )
            nc.vector.tensor_copy(R[:, 2:3], C2[:, 64:65])

            F = ppool.tile([1, 3], F32, tag="F")
            nc.tensor.matmul(F[:], ones_t, R[:], start=True, stop=True)
            Fs = pool.tile([1, 3], F32, tag="Fs")
            nc.scalar.copy(Fs[:], F[:])

            # num_graphs = max(batch) + 1, on device
            bmax = pool.tile([128, 1], F32, tag="bmax")
            nc.vector.tensor_reduce(bmax[:], bt, axis=AX.X, op=ALU.max)
            ball = pool.tile([128, 1], F32, tag="ball")
            nc.gpsimd.partition_all_reduce(
                ball[:], bmax[:], channels=128, reduce_op=bass_isa.ReduceOp.max
            )
            ng = pool.tile([1, 1], F32, tag="ng")
            nc.vector.tensor_scalar_add(ng[:], ball[0:1, 0:1], 1.0)
            rng = pool.tile([1, 1], F32, tag="rng")
            nc.vector.reciprocal(rng[:], ng[:])

            l2 = pool.tile([1, 1], F32, tag="l2")
            nc.vector.tensor_scalar(
                l2[:], Fs[:, 0:1], -float(PAD_NODES), PENALTY_SCALE / N_NODES,
                op0=ALU.add, op1=ALU.mult,
            )
            d32 = pool.tile([1, 1], F32, tag="d32")
            nc.vector.tensor_tensor(d32[:], Fs[:, 1:2], Fs[:, 2:3], op=ALU.subtract)
            t2s = pool.tile([1, 1], F32, tag="t2s")
            nc.vector.scalar_tensor_tensor(
                t2s[:], d32[:], 100.0, rng[:], op0=ALU.mult, op1=ALU.mult
            )
            res = pool.tile([1, 1], F32, tag="res")
            nc.vector.tensor_tensor(res[:], l2[:], t2s[:], op=ALU.add)
            nc.sync.dma_start(outd, res[:])

    nc.compile()
    return nc


def _pack_core(tt, uu, p, T):
    """Pack one core's edge shard into the [128, 7*T] fp32 edata layout."""
    ne = tt.shape[0]
    npad = T * 128

    def pad(a, fill):
        out = np.full(npad, fill, np.float64)
        out[:ne] = a
        return out.reshape(T, 128).T.astype(np.float32)  # [128, T]

    t_lo = pad(tt % 128, 0.0)
    t_hi = pad(tt // 128, float(HI))     # sentinel hi -> matches nothing
    u_lo = pad(uu % 128, 0.0)
    u_hi = pad(uu // 128, float(HI))
    tf = pad(tt, 0.0)
    uf = pad(uu, 0.0)                    # pad: tf==uf -> m=1, but p=0
    pf = pad(p, 0.0)
    return np.concatenate([t_lo, u_lo, t_hi, u_hi, tf, uf, pf], axis=1)


_CACHE = {}


def _get(name, builder, *a):
    if name not in _CACHE:
        _CACHE[name] = builder(*a)
    return _CACHE[name]


def kernel(x, edge_index, edge_feature, batch, _trace=False):
    x = np.asarray(x)
    ei = np.asarray(edge_index).astype(np.int64)
    p = np.asarray(edge_feature).astype(np.float32)[:, 0]
    batch = np.asarray(batch).astype(np.int64)

    uu_all = ei[0].astype(np.float64)
    tt_all = ei[1].astype(np.float64)

    iota128 = np.tile(np.arange(LO, dtype=np.float32), (128, 1))
    iota32 = np.tile(np.arange(HI, dtype=np.float32), (128, 1))
    ones = np.ones((128, 1), np.float32)
    cbias = np.zeros((128, 2), np.float32)
    cbias[:, 1] = 1.0 + 1e-6
    bpad = np.zeros(N_PAD, np.float32)
    bpad[:N_NODES] = batch.astype(np.float32)
    batchf = bpad.reshape(128, HI)

    # ---- phase 1: per-core partials (no cross-core dependencies)
    nc1 = _get("p1", _build_phase1, TPC)
    consts = np.concatenate([iota128, iota32, cbias], axis=1)
    in_maps = []
    for c in range(N_CORES):
        sl = slice(c * EPC, (c + 1) * EPC)
        edata = _pack_core(tt_all[sl], uu_all[sl], p[sl], TPC)
        in_maps.append({"blob": np.concatenate([consts, edata], axis=1)})
    r1 = bass_utils.run_bass_kernel_spmd(
        nc1, in_maps, core_ids=list(range(N_CORES)), trace=_trace
    )

    # gather/unshard the per-core partials (pure data movement)
    parts = np.concatenate(
        [np.asarray(r1.results[c]["partial"], np.float32) for c in range(N_CORES)],
        axis=1,
    )

    # ---- phase 2: combine on one core
    nc2 = _get("p2", _build_phase2)
    blob2 = np.concatenate([parts, ones, cbias, batchf], axis=1)
    r2 = bass_utils.run_bass_kernel_spmd(
        nc2, [{"blob2": blob2}], core_ids=[0], trace=_trace,
    )
    out = np.asarray(r2.results[0]["out"], dtype=np.float32).reshape(1, 1)
    if _trace:
        kernel.last_results = (r1, r2)
    return out
